# revision 4
# baseline (speedup 1.0000x reference)
"""Distributed causal multi-head attention for 8 TRN2 NeuronCores.

Problem: B=2, T=2048, D=1024, H=16 heads (hd=64), f32 in/out.

Sharding: core i handles batch b=i//4 and head-group g=i%4 (4 heads).
Wq/Wk/Wv are column-sharded ([1024, 256] per core), Wo row-sharded
([256, 1024] per core).  Each core computes a partial output projection
for its 4 heads over the full sequence; the host sums the 4 partials per
batch (the unshard step replaces the all-reduce).

Per-core dataflow (all matmuls bf16 on TensorEngine, f32 accumulation):
  x [2048,1024] --PE transpose--> xT bf16
  QT,KT [256(d),2048(t)] = W^T @ x^T   (d on partitions)
  V     [2048(t),256(d)]               (t on partitions, +ones col)
  ST[k,q] = K · Q^T  -> exp (ACT, scale=1/sqrt(64)) -> PT bf16
  causal mask on diagonal tiles via gpsimd affine_select
  AV: out[q, 65] += PT[k,q]^T @ Vaug[k, 65]  (col 64 = softmax denom)
  attn[q, dv] = AV[:, :64] * 1/AV[:, 64]     (per-partition scalar)
  attnT via PE transpose -> out_partial[t,e] = attnT^T @ Wo
"""

import numpy as np

import concourse.bass as bass
import concourse.mybir as mybir
import concourse.tile as tile
from concourse import bacc
from concourse.bass_utils import run_bass_kernel_spmd
from concourse.masks import make_identity

F32 = mybir.dt.float32
BF16 = mybir.dt.bfloat16

T = 2048  # sequence length
D = 1024  # embed dim
NH = 4  # heads per core
HD = 64  # head dim
DH = NH * HD  # 256, sharded d per core
TT = T // 128  # 16 t tiles
DT = D // 128  # 8 embed tiles
NSLAB = 4  # q slabs of 512
SCALE = 1.0 / np.sqrt(HD)

_NC_CACHE = None


def build():
    nc = bacc.Bacc(None, target_bir_lowering=False, debug=False)

    x = nc.declare_dram_parameter("x", [T, D], F32, isOutput=False)
    wq = nc.declare_dram_parameter("Wq", [D, DH], F32, isOutput=False)
    wk = nc.declare_dram_parameter("Wk", [D, DH], F32, isOutput=False)
    wv = nc.declare_dram_parameter("Wv", [D, DH], F32, isOutput=False)
    wo = nc.declare_dram_parameter("Wo", [DH, D], F32, isOutput=False)
    out = nc.declare_dram_parameter("out", [T, D], F32, isOutput=True)

    with tile.TileContext(nc) as tc:
        with (
            tc.tile_pool(name="persist", bufs=1) as persist,
            tc.tile_pool(name="xstage", bufs=3) as xstage_pool,
            tc.tile_pool(name="wstage", bufs=2) as wstage_pool,
            tc.tile_pool(name="pt", bufs=2) as pt_pool,
            tc.tile_pool(name="opev", bufs=3) as opev_pool,
            tc.tile_pool(name="recip", bufs=4) as recip_pool,
            tc.tile_pool(name="ps_big", bufs=2, space="PSUM") as ps_big,
            tc.tile_pool(name="ps_av", bufs=2, space="PSUM") as ps_av,
            tc.tile_pool(name="ps_tr", bufs=2, space="PSUM") as ps_tr,
        ):
            # ---- persistent SBUF tensors (distinct tags -> own slots) ----
            def P(shape, dtype, name):
                return persist.tile(shape, dtype, name=name, tag=name)

            ident_f = P([128, 128], F32, "ident_f")
            ident_b = P([128, 128], BF16, "ident_b")
            make_identity(nc, ident_f)
            make_identity(nc, ident_b)

            # weights, bf16: w*_bf[:, dt*256:(dt+1)*256] is D-tile dt
            wq_bf = P([128, DT * DH], BF16, "wq_bf")
            wk_bf = P([128, DT * DH], BF16, "wk_bf")
            wv_bf = P([128, DT * DH], BF16, "wv_bf")
            # wo_bf[:, i*1024:(i+1)*1024] is hdv-tile i
            wo_bf = P([128, 2 * D], BF16, "wo_bf")
            # xT[:, dt*2048 + t]: x transposed, bf16
            xT = P([128, DT * T], BF16, "xT")
            # QT/KT[:, m*2048 + t]: head h in tile h//2, rows (h%2)*64..+64
            QT = P([128, 2 * T], BF16, "QT")
            KT = P([128, 2 * T], BF16, "KT")
            # V with ones column: slice (tt, h) = [:, (tt*NH+h)*65 : +65]
            vbuf = P([128, TT * NH * 65], BF16, "vbuf")
            # attention output, natural: (qt, h) = [:, qt*256 + h*64]
            attn = P([128, TT * DH], BF16, "attn")
            # attn transposed: (i, t) = [:, i*2048 + t]
            attnT = P([128, 2 * T], BF16, "attnT")

            # ---- load + cast weights ----
            for w_ext, w_bf in ((wq, wq_bf), (wk, wk_bf), (wv, wv_bf)):
                for dt_ in range(DT):
                    ws = wstage_pool.tile([128, D], F32, name="ws")
                    nc.sync.dma_start(
                        out=ws[:, 0:DH], in_=w_ext[dt_ * 128 : (dt_ + 1) * 128, :]
                    )
                    nc.vector.tensor_copy(
                        w_bf[:, dt_ * DH : (dt_ + 1) * DH], ws[:, 0:DH]
                    )
            for i in range(2):
                ws = wstage_pool.tile([128, D], F32, name="ws")
                nc.sync.dma_start(out=ws[:], in_=wo[i * 128 : (i + 1) * 128, :])
                nc.vector.tensor_copy(wo_bf[:, i * D : (i + 1) * D], ws[:])

            # ones columns of vbuf (col 64 of each 65-block)
            vb3 = vbuf.rearrange("p (t c) -> p t c", c=65)
            nc.gpsimd.memset(vb3[:, :, 64:65], 1.0)

            # ---- x load + transpose (PE, fp32 2cyc/row) ----
            for tt in range(TT):
                xs = xstage_pool.tile([128, D], F32, name="xs")
                nc.sync.dma_start(out=xs[:], in_=x[tt * 128 : (tt + 1) * 128, :])
                for dt_ in range(DT):
                    ps = ps_tr.tile([128, 128], F32, name="pstr")
                    nc.tensor.transpose(
                        ps[:], xs[:, dt_ * 128 : (dt_ + 1) * 128], ident_f[:]
                    )
                    nc.scalar.copy(
                        out=xT[:, dt_ * T + tt * 128 : dt_ * T + (tt + 1) * 128],
                        in_=ps[:],
                    )

            # ---- QT / KT projections: out[d, t] (d on partitions) ----
            for w_bf, outT in ((wq_bf, QT), (wk_bf, KT)):
                for m in range(2):
                    for ch in range(4):  # t chunks of 512
                        ps = ps_big.tile([128, 512], F32, name="psbig")
                        for dt_ in range(DT):
                            nc.tensor.matmul(
                                ps[:],
                                lhsT=w_bf[
                                    :, dt_ * DH + m * 128 : dt_ * DH + (m + 1) * 128
                                ],
                                rhs=xT[:, dt_ * T + ch * 512 : dt_ * T + (ch + 1) * 512],
                                start=(dt_ == 0),
                                stop=(dt_ == DT - 1),
                            )
                        nc.vector.tensor_copy(
                            outT[:, m * T + ch * 512 : m * T + (ch + 1) * 512], ps[:]
                        )

            # ---- V projection: out[t, dv] (t on partitions) ----
            for tt in range(TT):
                ps = ps_big.tile([128, 512], F32, name="psbig")
                for dt_ in range(DT):
                    nc.tensor.matmul(
                        ps[:, 0:DH],
                        lhsT=xT[:, dt_ * T + tt * 128 : dt_ * T + (tt + 1) * 128],
                        rhs=wv_bf[:, dt_ * DH : (dt_ + 1) * DH],
                        start=(dt_ == 0),
                        stop=(dt_ == DT - 1),
                    )
                for h in range(NH):
                    nc.vector.tensor_copy(
                        vbuf[:, (tt * NH + h) * 65 : (tt * NH + h) * 65 + 64],
                        ps[:, h * 64 : (h + 1) * 64],
                    )

            # ---- attention: per (q-slab, head) ----
            for s in range(NSLAB):
                for h in range(NH):
                    m, r0 = h // 2, (h % 2) * 64
                    nk = 4 * (s + 1)  # k tiles in play for this slab
                    pt = pt_pool.tile([128, TT * 512], BF16, name="pt")
                    for kt in range(nk):
                        ps = ps_big.tile([128, 512], F32, name="psbig")
                        nc.tensor.matmul(
                            ps[:],
                            lhsT=KT[r0 : r0 + 64, m * T + kt * 128 : m * T + (kt + 1) * 128],
                            rhs=QT[r0 : r0 + 64, m * T + s * 512 : m * T + (s + 1) * 512],
                            start=True,
                            stop=True,
                        )
                        nc.scalar.activation(
                            out=pt[:, kt * 512 : (kt + 1) * 512],
                            in_=ps[:],
                            func=mybir.ActivationFunctionType.Exp,
                            scale=float(SCALE),
                        )
                        j = kt - 4 * s
                        if j >= 0:
                            # causal: keep where (s*512+qc) - (kt*128+kr) >= 0
                            nc.gpsimd.affine_select(
                                out=pt[:, kt * 512 : (kt + 1) * 512],
                                in_=pt[:, kt * 512 : (kt + 1) * 512],
                                pattern=[[1, 512]],
                                compare_op=mybir.AluOpType.is_ge,
                                fill=0.0,
                                base=-128 * j,
                                channel_multiplier=-1,
                            )
                    for qi in range(4):
                        qt = 4 * s + qi
                        av = ps_av.tile([128, 65], F32, name="psav")
                        for kt in range(qt + 1):
                            nc.tensor.matmul(
                                av[:],
                                lhsT=pt[:, kt * 512 + qi * 128 : kt * 512 + (qi + 1) * 128],
                                rhs=vbuf[:, (kt * NH + h) * 65 : (kt * NH + h + 1) * 65],
                                start=(kt == 0),
                                stop=(kt == qt),
                            )
                        rc = recip_pool.tile([128, 1], F32, name="rc")
                        nc.vector.reciprocal(rc[:], av[:, 64:65])
                        nc.vector.tensor_scalar_mul(
                            attn[:, qt * DH + h * 64 : qt * DH + (h + 1) * 64],
                            av[:, 0:64],
                            rc[:],
                        )

            # ---- transpose attn -> attnT ----
            for tt in range(TT):
                for i in range(2):
                    ps = ps_tr.tile([128, 128], BF16, name="pstrb")
                    nc.tensor.transpose(
                        ps[:],
                        attn[:, tt * DH + i * 128 : tt * DH + (i + 1) * 128],
                        ident_b[:],
                    )
                    nc.vector.tensor_copy(
                        attnT[:, i * T + tt * 128 : i * T + (tt + 1) * 128], ps[:]
                    )

            # ---- output projection (partial; host sums across 4 cores) ----
            for tt in range(TT):
                for ec in range(2):
                    ps = ps_big.tile([128, 512], F32, name="psbig")
                    for i in range(2):
                        nc.tensor.matmul(
                            ps[:],
                            lhsT=attnT[:, i * T + tt * 128 : i * T + (tt + 1) * 128],
                            rhs=wo_bf[:, i * D + ec * 512 : i * D + (ec + 1) * 512],
                            start=(i == 0),
                            stop=(i == 1),
                        )
                    ev = opev_pool.tile([128, 512], F32, name="ev")
                    nc.vector.tensor_copy(ev[:], ps[:])
                    nc.sync.dma_start(
                        out=out[tt * 128 : (tt + 1) * 128, ec * 512 : (ec + 1) * 512],
                        in_=ev[:],
                    )

    nc.compile()
    return nc


def _get_nc():
    global _NC_CACHE
    if _NC_CACHE is None:
        _NC_CACHE = build()
    return _NC_CACHE


def make_in_maps(x, Wq, Wk, Wv, Wo):
    x = np.asarray(x, dtype=np.float32)
    Wq = np.asarray(Wq, dtype=np.float32)
    Wk = np.asarray(Wk, dtype=np.float32)
    Wv = np.asarray(Wv, dtype=np.float32)
    Wo = np.asarray(Wo, dtype=np.float32)
    in_maps = []
    for core in range(8):
        b, g = core // 4, core % 4
        sl = slice(g * DH, (g + 1) * DH)
        in_maps.append(
            {
                "x": np.ascontiguousarray(x[b]),
                "Wq": np.ascontiguousarray(Wq[:, sl]),
                "Wk": np.ascontiguousarray(Wk[:, sl]),
                "Wv": np.ascontiguousarray(Wv[:, sl]),
                "Wo": np.ascontiguousarray(Wo[sl, :]),
            }
        )
    return in_maps


def unshard(results):
    out = np.empty((2, T, D), np.float32)
    for b in range(2):
        out[b] = results[4 * b]["out"]
        for g in range(1, 4):
            out[b] += results[4 * b + g]["out"]
    return out


def kernel(x, Wq, Wk, Wv, Wo):
    nc = _get_nc()
    in_maps = make_in_maps(x, Wq, Wk, Wv, Wo)
    res = run_bass_kernel_spmd(nc, in_maps, core_ids=list(range(8)))
    return unshard(res.results)


# revision 5
# speedup vs baseline: 1.2041x; 1.2041x over previous
"""Distributed causal multi-head attention for 8 TRN2 NeuronCores.

Problem: B=2, T=2048, D=1024, H=16 heads (hd=64), f32 in/out.

Sharding: core i handles batch b=i//4 and head-group g=i%4 (4 heads).
Wq/Wk/Wv are column-sharded ([1024, 256] per core), Wo row-sharded
([256, 1024] per core).  Each core computes a partial output projection
for its 4 heads over the full sequence; the host sums the 4 partials per
batch (the unshard step replaces the all-reduce).

Per-core dataflow (matmuls bf16 on TensorEngine, f32 accumulation):
  x [2048,1024] --PE transpose--> xT bf16
  QT,KT [256(d),2048(t)] = W^T @ x^T   (d on partitions)
  V     [2048(t),256(d)]               (t on partitions, +ones col)
  ST[k,q] = K . Q^T  -> exp (ACT, scale=1/sqrt(64)) -> PT bf16
  causal: diagonal tiles narrowed to their valid q range; only the
  128-wide diagonal block needs an affine_select mask (gpsimd)
  AV: out[q, 65] += PT[k,q]^T @ Vaug[k, 65]  (col 64 = softmax denom)
  attn[q, dv] = AV[:, :64] * recip(AV[:, 64])  (ACT copy w/ scale)
  attnT via PE transpose -> out_partial[t,e] = attnT^T @ Wo

Emission is software-pipelined: scores of head-pair p interleave with
AV of pair p-1, and each q-slab's epilogue (transpose + out-proj + DMA)
is emitted as soon as the slab completes, keeping the PE stream dense
(HAM stays warm).
"""

import numpy as np

import concourse.bass as bass
import concourse.mybir as mybir
import concourse.tile as tile
from concourse import bacc
from concourse.bass_utils import run_bass_kernel_spmd
from concourse.masks import make_identity

F32 = mybir.dt.float32
BF16 = mybir.dt.bfloat16
AF = mybir.ActivationFunctionType

T = 2048  # sequence length
D = 1024  # embed dim
NH = 4  # heads per core
HD = 64  # head dim
DH = NH * HD  # 256, sharded d per core
TT = T // 128  # 16 t tiles
DT = D // 128  # 8 embed tiles
NSLAB = 4  # q slabs of 512
SCALE = 1.0 / np.sqrt(HD)

_NC_CACHE = None


def build():
    nc = bacc.Bacc(None, target_bir_lowering=False, debug=False)

    x = nc.declare_dram_parameter("x", [T, D], F32, isOutput=False)
    wq = nc.declare_dram_parameter("Wq", [D, DH], F32, isOutput=False)
    wk = nc.declare_dram_parameter("Wk", [D, DH], F32, isOutput=False)
    wv = nc.declare_dram_parameter("Wv", [D, DH], F32, isOutput=False)
    wo = nc.declare_dram_parameter("Wo", [DH, D], F32, isOutput=False)
    out = nc.declare_dram_parameter("out", [T, D], F32, isOutput=True)

    with tile.TileContext(nc) as tc:
        with (
            tc.tile_pool(name="persist", bufs=1) as persist,
            tc.tile_pool(name="xstage", bufs=3) as xstage_pool,
            tc.tile_pool(name="wstage", bufs=2) as wstage_pool,
            tc.tile_pool(name="pt", bufs=2) as pt_pool,
            tc.tile_pool(name="opev", bufs=3) as opev_pool,
            tc.tile_pool(name="recip", bufs=4) as recip_pool,
            tc.tile_pool(name="ps_st", bufs=2, space="PSUM") as ps_st,
            tc.tile_pool(name="ps_av", bufs=2, space="PSUM") as ps_av,
            tc.tile_pool(name="ps_tr", bufs=2, space="PSUM") as ps_tr,
        ):
            # ---- persistent SBUF tensors (distinct tags -> own slots) ----
            def P(shape, dtype, name):
                return persist.tile(shape, dtype, name=name, tag=name)

            ident_f = P([128, 128], F32, "ident_f")
            ident_b = P([128, 128], BF16, "ident_b")
            make_identity(nc, ident_f)
            make_identity(nc, ident_b)

            # weights, bf16: w*_bf[:, dt*256:(dt+1)*256] is D-tile dt
            wq_bf = P([128, DT * DH], BF16, "wq_bf")
            wk_bf = P([128, DT * DH], BF16, "wk_bf")
            wv_bf = P([128, DT * DH], BF16, "wv_bf")
            # wo_bf[:, i*1024:(i+1)*1024] is hdv-tile i
            wo_bf = P([128, 2 * D], BF16, "wo_bf")
            # xT[:, dt*2048 + t]: x transposed, bf16
            xT = P([128, DT * T], BF16, "xT")
            # QT/KT[:, m*2048 + t]: head h in tile h//2, rows (h%2)*64..+64
            QT = P([128, 2 * T], BF16, "QT")
            KT = P([128, 2 * T], BF16, "KT")
            # V with ones column: slice (tt, h) = [:, (tt*NH+h)*65 : +65]
            vbuf = P([128, TT * NH * 65], BF16, "vbuf")
            # attention output, natural: (qt, h) = [:, qt*256 + h*64]
            attn = P([128, TT * DH], BF16, "attn")
            # attn transposed: (i, t) = [:, i*2048 + t]
            attnT = P([128, 2 * T], BF16, "attnT")

            # ---- load + cast weights (casts on gpsimd: SBUF->SBUF) ----
            for w_ext, w_bf in ((wq, wq_bf), (wk, wk_bf), (wv, wv_bf)):
                for dt_ in range(DT):
                    ws = wstage_pool.tile([128, D], F32, name="ws")
                    nc.sync.dma_start(
                        out=ws[:, 0:DH], in_=w_ext[dt_ * 128 : (dt_ + 1) * 128, :]
                    )
                    nc.gpsimd.tensor_copy(
                        w_bf[:, dt_ * DH : (dt_ + 1) * DH], ws[:, 0:DH]
                    )
            for i in range(2):
                ws = wstage_pool.tile([128, D], F32, name="ws")
                nc.sync.dma_start(out=ws[:], in_=wo[i * 128 : (i + 1) * 128, :])
                nc.gpsimd.tensor_copy(wo_bf[:, i * D : (i + 1) * D], ws[:])

            # ones columns of vbuf (col 64 of each 65-block)
            vb3 = vbuf.rearrange("p (t c) -> p t c", c=65)
            nc.gpsimd.memset(vb3[:, :, 64:65], 1.0)

            xT3 = xT.rearrange("p (d t) -> p d t", d=DT)

            def emit_x_transposes(tts):
                # 4 transposes packed per PSUM bank, one strided DVE evac
                for tt in tts:
                    xs = xstage_pool.tile([128, D], F32, name="xs")
                    nc.sync.dma_start(
                        out=xs[:], in_=x[tt * 128 : (tt + 1) * 128, :]
                    )
                    for g4 in range(2):
                        ps = ps_tr.tile([128, 512], F32, name="pstr", tag="pstr")
                        for u in range(4):
                            dt_ = g4 * 4 + u
                            nc.tensor.transpose(
                                ps[:, u * 128 : (u + 1) * 128],
                                xs[:, dt_ * 128 : (dt_ + 1) * 128],
                                ident_f[:],
                            )
                        nc.vector.tensor_copy(
                            xT3[
                                :, g4 * 4 : (g4 + 1) * 4, tt * 128 : (tt + 1) * 128
                            ],
                            ps.rearrange("p (u c) -> p u c", u=4),
                        )

            def emit_qk_proj(ch2):
                # QT/KT columns [ch2*1024, +1024]
                for w_bf, outT in ((wq_bf, QT), (wk_bf, KT)):
                    for m in range(2):
                        ps = ps_st.tile([128, 1024], F32, name="psst")
                        for dt_ in range(DT):
                            lhsT = w_bf[
                                :, dt_ * DH + m * 128 : dt_ * DH + (m + 1) * 128
                            ]
                            for half in range(2):
                                c0 = ch2 * 1024 + half * 512
                                nc.tensor.matmul(
                                    ps[:, half * 512 : (half + 1) * 512],
                                    lhsT=lhsT,
                                    rhs=xT[:, dt_ * T + c0 : dt_ * T + c0 + 512],
                                    start=(dt_ == 0),
                                    stop=(dt_ == DT - 1),
                                )
                        nc.scalar.copy(
                            out=outT[:, m * T + ch2 * 1024 : m * T + (ch2 + 1) * 1024],
                            in_=ps[:],
                        )

            vb4 = vbuf.rearrange("p (n c) -> p n c", c=65)

            def emit_v_proj(tts):
                for tt in tts:
                    ps = ps_av.tile([128, 256], F32, name="psav", tag="psav")
                    for dt_ in range(DT):
                        nc.tensor.matmul(
                            ps[:],
                            lhsT=xT[:, dt_ * T + tt * 128 : dt_ * T + (tt + 1) * 128],
                            rhs=wv_bf[:, dt_ * DH : (dt_ + 1) * DH],
                            start=(dt_ == 0),
                            stop=(dt_ == DT - 1),
                        )
                    nc.vector.tensor_copy(
                        vb4[:, tt * NH : (tt + 1) * NH, 0:64],
                        ps.rearrange("p (n c) -> p n c", n=NH),
                    )

            # ---- attention emission helpers ----
            def scores_chunks(s, h, pt):
                """List of thunks; each computes scores+exp for 1-2 k-tiles."""
                m, r0 = h // 2, (h % 2) * 64

                def off_diag(kt):
                    def go():
                        ps = ps_st.tile([128, 1024], F32, name="psst")
                        for u in range(2):
                            nc.tensor.matmul(
                                ps[:, u * 512 : (u + 1) * 512],
                                lhsT=KT[
                                    r0 : r0 + 64,
                                    m * T + (kt + u) * 128 : m * T + (kt + u + 1) * 128,
                                ],
                                rhs=QT[
                                    r0 : r0 + 64,
                                    m * T + s * 512 : m * T + (s + 1) * 512,
                                ],
                                start=True,
                                stop=True,
                            )
                        nc.scalar.activation(
                            out=pt[:, kt * 512 : (kt + 2) * 512],
                            in_=ps[:],
                            func=AF.Exp,
                            scale=float(SCALE),
                        )

                    return go

                def diag(j):
                    kt = 4 * s + j
                    n = 512 - 128 * j

                    def go():
                        ps = ps_st.tile([128, 1024], F32, name="psst")
                        nc.tensor.matmul(
                            ps[:, 0:n],
                            lhsT=KT[
                                r0 : r0 + 64, m * T + kt * 128 : m * T + (kt + 1) * 128
                            ],
                            rhs=QT[
                                r0 : r0 + 64,
                                m * T + s * 512 + 128 * j : m * T + (s + 1) * 512,
                            ],
                            start=True,
                            stop=True,
                        )
                        nc.scalar.activation(
                            out=pt[:, kt * 512 + 128 * j : (kt + 1) * 512],
                            in_=ps[:, 0:n],
                            func=AF.Exp,
                            scale=float(SCALE),
                        )
                        # mask the 128-wide diagonal block: keep qc_local>=kr
                        nc.gpsimd.affine_select(
                            out=pt[:, kt * 512 + 128 * j : kt * 512 + 128 * (j + 1)],
                            in_=pt[:, kt * 512 + 128 * j : kt * 512 + 128 * (j + 1)],
                            pattern=[[1, 128]],
                            compare_op=mybir.AluOpType.is_ge,
                            fill=0.0,
                            base=0,
                            channel_multiplier=-1,
                        )

                    return go

                return [off_diag(2 * u) for u in range(2 * s)] + [
                    diag(j) for j in range(4)
                ]

            def av_ops(s, h, pt):
                """List of thunks; each computes AV + normalize for one q tile."""
                ops = []
                for qi in range(4):
                    qt = 4 * s + qi

                    def go(qi=qi, qt=qt):
                        av = ps_av.tile([128, 256], F32, name="psav", tag="psav")
                        for kt in range(qt + 1):
                            nc.tensor.matmul(
                                av[:, 0:65],
                                lhsT=pt[
                                    :, kt * 512 + qi * 128 : kt * 512 + (qi + 1) * 128
                                ],
                                rhs=vb4[:, kt * NH + h, :],
                                start=(kt == 0),
                                stop=(kt == qt),
                            )
                        rc = recip_pool.tile([128, 1], F32, name="rc")
                        nc.vector.reciprocal(rc[:], av[:, 64:65])
                        nc.scalar.activation(
                            out=attn[:, qt * DH + h * 64 : qt * DH + (h + 1) * 64],
                            in_=av[:, 0:64],
                            func=AF.Copy,
                            scale=rc[:],
                        )

                    ops.append(go)
                return ops

            at3 = attnT.rearrange("p (i t) -> p i t", i=2)

            def emit_slab_epilogue(s):
                for qt in range(4 * s, 4 * (s + 1)):
                    ps = ps_tr.tile([128, 256], BF16, name="pstrb", tag="pstr")
                    for i in range(2):
                        nc.tensor.transpose(
                            ps[:, i * 128 : (i + 1) * 128],
                            attn[:, qt * DH + i * 128 : qt * DH + (i + 1) * 128],
                            ident_b[:],
                        )
                    nc.vector.tensor_copy(
                        at3[:, :, qt * 128 : (qt + 1) * 128],
                        ps.rearrange("p (i c) -> p i c", i=2),
                    )
                for tt in range(4 * s, 4 * (s + 1)):
                    ps = ps_st.tile([128, 1024], F32, name="psst")
                    for i in range(2):
                        lhsT = attnT[:, i * T + tt * 128 : i * T + (tt + 1) * 128]
                        for ec in range(2):
                            nc.tensor.matmul(
                                ps[:, ec * 512 : (ec + 1) * 512],
                                lhsT=lhsT,
                                rhs=wo_bf[:, i * D + ec * 512 : i * D + (ec + 1) * 512],
                                start=(i == 0),
                                stop=(i == 1),
                            )
                    ev = opev_pool.tile([128, 1024], F32, name="ev")
                    nc.vector.tensor_copy(ev[:], ps[:])
                    nc.sync.dma_start(
                        out=out[tt * 128 : (tt + 1) * 128, :], in_=ev[:]
                    )

            def interleave(a, b):
                """Merge op lists proportionally (a paced, b filled in)."""
                if not a:
                    return list(b)
                if not b:
                    return list(a)
                res = []
                nb, na, bi = len(b), len(a), 0
                for i, op in enumerate(a):
                    res.append(op)
                    want = (i + 1) * nb // na
                    while bi < want:
                        res.append(b[bi])
                        bi += 1
                res.extend(b[bi:])
                return res

            # ---- phase 0/1: x transposes + projections ----
            emit_x_transposes(range(0, 8))
            emit_qk_proj(0)
            emit_x_transposes(range(8, 16))
            emit_qk_proj(1)
            emit_v_proj(range(0, TT))

            # ---- attention, software-pipelined by one head-pair ----
            pairs = [(s, h) for s in range(NSLAB) for h in range(NH)]
            pts = {}
            prev = None
            for idx in range(len(pairs) + 1):
                sc = []
                if idx < len(pairs):
                    s, h = pairs[idx]
                    pts[idx] = pt_pool.tile([128, TT * 512], BF16, name="pt")
                    sc = scores_chunks(s, h, pts[idx])
                av = []
                if prev is not None:
                    ps_, ph_ = pairs[prev]
                    av = av_ops(ps_, ph_, pts[prev])
                for op in interleave(sc, av):
                    op()
                if prev is not None and pairs[prev][1] == NH - 1:
                    emit_slab_epilogue(pairs[prev][0])
                prev = idx

    nc.compile()
    return nc


def _get_nc():
    global _NC_CACHE
    if _NC_CACHE is None:
        _NC_CACHE = build()
    return _NC_CACHE


def make_in_maps(x, Wq, Wk, Wv, Wo):
    x = np.asarray(x, dtype=np.float32)
    Wq = np.asarray(Wq, dtype=np.float32)
    Wk = np.asarray(Wk, dtype=np.float32)
    Wv = np.asarray(Wv, dtype=np.float32)
    Wo = np.asarray(Wo, dtype=np.float32)
    in_maps = []
    for core in range(8):
        b, g = core // 4, core % 4
        sl = slice(g * DH, (g + 1) * DH)
        in_maps.append(
            {
                "x": np.ascontiguousarray(x[b]),
                "Wq": np.ascontiguousarray(Wq[:, sl]),
                "Wk": np.ascontiguousarray(Wk[:, sl]),
                "Wv": np.ascontiguousarray(Wv[:, sl]),
                "Wo": np.ascontiguousarray(Wo[sl, :]),
            }
        )
    return in_maps


def unshard(results):
    out = np.empty((2, T, D), np.float32)
    for b in range(2):
        out[b] = results[4 * b]["out"]
        for g in range(1, 4):
            out[b] += results[4 * b + g]["out"]
    return out


def kernel(x, Wq, Wk, Wv, Wo):
    nc = _get_nc()
    in_maps = make_in_maps(x, Wq, Wk, Wv, Wo)
    res = run_bass_kernel_spmd(nc, in_maps, core_ids=list(range(8)))
    return unshard(res.results)
